# revision 11
# baseline (speedup 1.0000x reference)
"""Windowed multi-head attention TRN2 kernel (Bass/Tile), SPMD over 8 cores.

Problem (per reference): x:(8,512,64,64) viewed as (B, 4096 tok, 512 c);
Q/K/V = tok @ W^T + b; per window (64 tok) & head (8 x 64d):
softmax(QK^T/8 + Bbias) @ V; output back in (B,512,64,64).

Sharding: data-parallel, one batch element per core (8 cores).

Per-core dataflow (fp16 matmul operands, fp32 PSUM accum, fp16 HBM out):
 - host passes x^T (c, tok) fp16; one 3-D-AP DMA loads all 4 channel
   chunks of a 512-token tile
 - Q^T/K^T projections evacuate naturally on ACT, then DVE strided
   copies build per-head STACKED (Qs) and BLOCK-DIAGONAL (Kd) operands;
   off-diagonal zeros live in persistent tiles, written once (all
   partition-shifted copies run on DVE — NEVER GpSimd/Pool: its 8 DSPs
   own fixed 16-partition slices and cross-partition traffic crawls)
 - scores: per (window-pair, head) ONE full-128 matmul
   s[:, h] = Kd_h^T @ Qs_h stacks both windows' scores^T k-major;
   Bbias^T is pre-seeded into the PSUM bank by an [I|I]^T @ BbiasT
   matmul (start=True) and QK accumulates on top (start=False,
   stop only on the last head - stop clears the whole zero-region)
 - exp on ACT writes straight into the pre-zeroed block-diag ptd tile
   (two strided half-partition activations)
 - PV: ONE full-128 matmul per head: o = ptd_h^T @ V_h; V natural
   [tok, c] with a per-head ones-column (65-wide blocks, written once
   into persistent tiles) so PV also emits softmax denominators
 - scores(tt+1), scores(tt+2) are issued on the PE before PV(tt)
   (software pipelining) so ACT exp runs in the PE queue's shadow
 - normalize: per 4-head group one strided reciprocal + one strided
   broadcast multiply on DVE; fp16 [tok, c] tiles stored to HBM
"""

import sys
import numpy as np

for _p in ("/opt/trn_rl_repo",):
    if _p not in sys.path:
        sys.path.insert(0, _p)

from contextlib import ExitStack

import concourse.bass as bass
import concourse.tile as tile
from concourse import mybir

F16 = mybir.dt.float16
F32 = mybir.dt.float32

B, C, HH, WW = 8, 512, 64, 64
NH, HD = 8, 64
WIN = 64            # tokens per window
TOK = C * 0 + 4096  # tokens per batch/core
NT = 8              # 512-token tiles per core
NCHUNK = 4          # 128-channel chunks

TRACE = False
LAST = {}


def _emit(tc, out, xT, wq, wk, wv, ebt, iit, bqk):
    """Emit the per-core program. bqk: [128, 8] fp32 (bq/8 | bk chunks) or None.

    Block-diagonal dataflow: per (window-pair tt, head h), scores and PV are
    ONE full-128-partition matmul each. The stationary operands are built
    block-diagonally (zeros off-diagonal, pre-zeroed once per pool buffer at
    startup, diagonal blocks rewritten each iteration):
      Kd_h  [128 ch|ch, 4tt x (k(w0)|k(w1))]  - K^T diag per window pair
      Qs_h  [128 ch|ch, 4tt x 64q]            - Q^T stacked (w0 top, w1 bot)
      ptd   [128 k|k,  8h x (q(w0)|q(w1))]    - exp(scores^T)*ebt diag
    so  s[:, h] = Kd_h(tt)^T @ Qs_h(tt)  gives both windows' scores^T stacked
    and o[:, hh] = ptd(h)^T @ V(h) gives both windows' outputs stacked —
    token-major, normalized by one batched reciprocal+mul per 4-head group.
    """
    nc = tc.nc
    Exp = mybir.ActivationFunctionType.Exp
    Ident = mybir.ActivationFunctionType.Identity

    with ExitStack() as ctx:
        ep = ctx.enter_context

        wpool = ep(tc.tile_pool(name="w", bufs=1))
        xpool = ep(tc.tile_pool(name="x", bufs=2))
        qkpool = ep(tc.tile_pool(name="qk", bufs=2))
        vpool = ep(tc.tile_pool(name="v", bufs=2))
        epool = ep(tc.tile_pool(name="e", bufs=2))
        rcpool = ep(tc.tile_pool(name="rc", bufs=4))
        onpool = ep(tc.tile_pool(name="on", bufs=3))
        projps = ep(tc.tile_pool(name="projps", bufs=3, space="PSUM"))
        sps = ep(tc.tile_pool(name="sps", bufs=3, space="PSUM"))
        ops = ep(tc.tile_pool(name="ops", bufs=2, space="PSUM"))

        # resident weights: [c_in chunk 128, c_out 512] fp16 per proj
        wsb = {}
        for nm, wdram in (("q", wq), ("k", wk), ("v", wv)):
            for ci in range(NCHUNK):
                t = wpool.tile([128, 512], F16, tag=f"w{nm}{ci}")
                nc.sync.dma_start(t[:], wdram[ci * 128:(ci + 1) * 128, :])
                wsb[nm, ci] = t
        # ebt here is raw Bbias^T (k-rows duplicated) tiled across the 8
        # head column-blocks: [128, 512] f16, preloaded into the scores
        # PSUM bank each subtile so the QK matmuls accumulate on top of it.
        ebt_sb = wpool.tile([128, 512], F16, tag="ebt")
        nc.sync.dma_start(ebt_sb[:], ebt[:, :])
        bqk_sb = None
        if bqk is not None:
            bqk_sb = wpool.tile([128, 8], F32, tag="bqk")
            nc.sync.dma_start(bqk_sb[:], bqk[:, :])

        # persistent double-buffered block-diagonal tiles (allocated once so
        # the zero regions survive across iterations; diagonal blocks are
        # rewritten each use, off-diagonal zeros written once here).
        kdbuf = [[wpool.tile([128, 512], F16, tag=f"kd{b}{h}",
                             name=f"kd{b}{h}")
                  for h in range(8)] for b in range(2)]
        qsbuf = [[wpool.tile([128, 256], F16, tag=f"qs{b}{h}",
                             name=f"qs{b}{h}")
                  for h in range(8)] for b in range(2)]
        ptdbuf = [wpool.tile([128, 1024], F16, tag=f"ptd{b}",
                             name=f"ptd{b}")
                  for b in range(3)]
        for b in range(2):
            for h in range(8):
                nc.vector.memset(kdbuf[b][h][:], 0.0)
        for b in range(3):
            nc.vector.memset(ptdbuf[b][:], 0.0)

        # [I64 | I64] stationary: one matmul per subtile seeds the scores
        # PSUM bank with the Bbias^T pattern (out[k, h*64+q] = ebt[k%64, q])
        iit_sb = wpool.tile([64, 128], F16, tag="iit")
        nc.sync.dma_start(iit_sb[:], iit[:, :])

        # persistent V tiles: the per-head ones column (col 64 of each
        # 65-block) is written once here and never touched again; the
        # in-loop evacuation only writes cols 0:64 of each block.
        vnbuf = [[wpool.tile([128, 520], F16, tag=f"vn{b}{tt}",
                             name=f"vn{b}{tt}")
                  for tt in range(NCHUNK)] for b in range(2)]
        for b in range(2):
            for tt in range(NCHUNK):
                vv = vnbuf[b][tt][:].rearrange("p (h x) -> p h x", x=65)
                nc.scalar.activation(
                    vv[:, :, 64], ebt_sb[:, 0:8], Ident,
                    bias=1.0, scale=0.0)

        for T in range(NT):
            # ---- load all four x^T chunks [c_in 128, tok 512] in one DMA
            xt_all = xpool.tile([128, 2048], F16, tag="xt")
            nc.sync.dma_start(
                xt_all[:].rearrange("p (c t) -> p c t", t=512),
                xT[:, T * 512:(T + 1) * 512].rearrange(
                    "(c p) t -> p c t", p=128))


            # ---- Q^T / K^T projections: natural full-width ACT evacuation,
            # then f16 strided DVE copies into the stacked / diagonal
            # layouts.
            qs, kd = qsbuf[T % 2], kdbuf[T % 2]
            for pi, nm in enumerate(("q", "k")):
                for co in range(NCHUNK):
                    ps = projps.tile([128, 512], F32, tag="proj")
                    for ci in range(NCHUNK):
                        nc.tensor.matmul(
                            ps[:],
                            wsb[nm, ci][:, co * 128:(co + 1) * 128],
                            xt_all[:, ci * 512:(ci + 1) * 512],
                            start=(ci == 0), stop=(ci == NCHUNK - 1))
                    t = qkpool.tile([128, 512], F16, tag=f"{nm}t{co}")
                    if bqk_sb is not None:
                        nc.scalar.activation(
                            t[:], ps[:], Ident,
                            bias=bqk_sb[:, pi * 4 + co:pi * 4 + co + 1])
                    else:
                        nc.scalar.copy(t[:], ps[:])
                    # natural cols viewed as (tt, parity, 64tok)
                    t_v = t[:].rearrange("p (t two x) -> p t two x", two=2, x=64)
                    for e in range(2):
                        h = 2 * co + e
                        er = slice(e * 64, e * 64 + 64)
                        if nm == "q":
                            d = qs[h]
                            dsts = (
                                d[0:64, :].rearrange("p (t x) -> p t x", x=64),
                                d[64:128, :].rearrange("p (t x) -> p t x", x=64))
                            eng = nc.vector
                        else:
                            d = kd[h]
                            dv = d[:].rearrange("p (t x) -> p t x", x=128)
                            dsts = (dv[0:64, :, 0:64], dv[64:128, :, 64:128])
                            eng = nc.vector
                        for p in range(2):
                            eng.tensor_copy(dsts[p], t_v[er, :, p, :])

            # ---- V natural projection per 128-tok subtile -> [tok 128,
            # 8 x (64 d | 1)] with a ones column per head for softmax sums
            vnat = []
            for tt in range(NCHUNK):
                ps = projps.tile([128, 512], F32, tag="proj")
                for ci in range(NCHUNK):
                    nc.tensor.matmul(
                        ps[:],
                        xt_all[:, ci * 512 + tt * 128:
                               ci * 512 + (tt + 1) * 128],
                        wsb["v", ci][:],
                        start=(ci == 0), stop=(ci == NCHUNK - 1))
                vn = vnbuf[T % 2][tt]
                vn_v = vn[:].rearrange("p (h x) -> p h x", x=65)
                nc.scalar.copy(
                    vn_v[:, :, 0:64],
                    ps[:].rearrange("p (h x) -> p h x", x=64))
                vnat.append(vn)

            # ---- attention: subtile tt covers windows 2tt (partitions
            # 0:64) and 2tt+1 (64:128); one full-128 matmul per head for
            # scores and for PV via the block-diagonal stationaries.
            # Software-pipelined: scores(tt+1), scores(tt+2) are issued on
            # the PE BEFORE PV(tt), so the exp stage (ACT, written straight
            # into the diag tile) runs in the shadow of PE work instead of
            # stalling the in-order PE queue at every handoff.
            def emit_scores(tt):
                s = sps.tile([128, 512], F32, tag="s")
                # seed full bank with Bbias^T, then accumulate QK on top
                nc.tensor.matmul(s[:], iit_sb[:], ebt_sb[0:64, :],
                                 start=True, stop=False)
                for h in range(8):
                    nc.tensor.matmul(
                        s[:, h * 64:(h + 1) * 64],
                        kd[h][:, tt * 128:(tt + 1) * 128],
                        qs[h][:, tt * 64:(tt + 1) * 64],
                        start=False, stop=(h == 7))
                ptd = ptdbuf[tt % 3]
                ptd_v = ptd[:].rearrange("p (h x) -> p h x", x=128)
                for p in range(2):
                    r = slice(p * 64, p * 64 + 64)
                    nc.scalar.activation(
                        ptd_v[r, :, p * 64:(p + 1) * 64],
                        s[r, :].rearrange("p (h x) -> p h x", x=64), Exp)
                return ptd

            ptds = [emit_scores(0), emit_scores(1)]
            for tt in range(NCHUNK):
                if tt + 2 < NCHUNK:
                    ptds.append(emit_scores(tt + 2))
                ptd = ptds[tt]
                # PV into two 4-head PSUM tiles [128, 4 x (64 d | 1 sum)]
                on = onpool.tile([128, 512], F16, tag="on")
                rc = rcpool.tile([128, 8], F32, tag="rc")
                for g in range(2):
                    o = ops.tile([128, 512], F32, tag="o")
                    o_v = o[:, 0:260].rearrange("p (h x) -> p h x", x=65)
                    for hh in range(4):
                        h = g * 4 + hh
                        nc.tensor.matmul(
                            o[:, hh * 65:(hh + 1) * 65],
                            ptd[:, h * 128:(h + 1) * 128],
                            vnat[tt][:, h * 65:(h + 1) * 65],
                            start=True, stop=True)
                    nc.vector.reciprocal(
                        rc[:, g * 4:(g + 1) * 4], o_v[:, :, 64])
                    nc.vector.tensor_mul(
                        on[:, g * 256:(g + 1) * 256].rearrange(
                            "p (h x) -> p h x", x=64),
                        o_v[:, :, 0:64],
                        rc[:, g * 4:(g + 1) * 4].unsqueeze(2).broadcast_to(
                            (128, 4, 64)))
                nc.sync.dma_start(
                    out[T * 512 + tt * 128: T * 512 + (tt + 1) * 128, :], on[:])


def _legalize_sync(nc, max_waits=1):
    """Hoist excess semaphore waits into standalone same-engine
    EventSemaphore instructions. Engine instruction streams execute in
    order, so a wait carried by an immediately-preceding EventSemaphore is
    equivalent to a wait on the instruction itself — and the walrus build
    in this environment rejects instructions with more than one wait."""
    import bass_rust
    n_new = 0
    fn = nc.m.functions[0]
    for blk in fn.blocks:
        out = []
        changed = False
        for ins in blk.instructions:
            si = ins.sync_info
            waits = list(si.on_wait) if si and si.on_wait else []
            if len(waits) > max_waits:
                keep = waits[-max_waits:]
                for w in waits[:-max_waits]:
                    es = mybir.InstEventSemaphore(
                        name=f"esw-{n_new}-{ins.name}", ins=[], outs=[])
                    es.engine = ins.engine
                    es.sync_info = bass_rust.SyncInfo(on_wait=[w], on_update=[])
                    out.append(es)
                    n_new += 1
                ins.sync_info = bass_rust.SyncInfo(
                    on_wait=keep,
                    on_update=list(si.on_update) if si.on_update else [])
                changed = True
            out.append(ins)
        if changed:
            blk.instructions = out
    return n_new


def _build_model(with_bias, reps=1):
    nc = bass.Bass("TRN2", target_bir_lowering=False, debug=False)
    xT = nc.dram_tensor("xT", [512, 4096], F16, kind="ExternalInput").ap()
    wq = nc.dram_tensor("wq", [512, 512], F16, kind="ExternalInput").ap()
    wk = nc.dram_tensor("wk", [512, 512], F16, kind="ExternalInput").ap()
    wv = nc.dram_tensor("wv", [512, 512], F16, kind="ExternalInput").ap()
    ebt = nc.dram_tensor("ebt", [128, 512], F16, kind="ExternalInput").ap()
    iit = nc.dram_tensor("iit", [64, 128], F16, kind="ExternalInput").ap()
    bqk = (nc.dram_tensor("bqk", [128, 8], F32, kind="ExternalInput").ap()
           if with_bias else None)
    outs = [nc.dram_tensor("out" if r == 0 else f"out{r}", [4096, 512], F16,
                           kind="ExternalOutput").ap() for r in range(reps)]
    with tile.TileContext(nc) as tc:
        for r in range(reps):
            _emit(tc, outs[r], xT, wq, wk, wv, ebt, iit, bqk)
    return nc


_MODEL_CACHE = {}


def get_model(with_bias=False, legalize=True, reps=1):
    key = (with_bias, legalize, reps)
    if key not in _MODEL_CACHE:
        nc = _build_model(with_bias, reps=reps)
        if legalize:
            _legalize_sync(nc)
        _MODEL_CACHE[key] = nc
    return _MODEL_CACHE[key]


def make_in_maps(x, Wq, bq, Wk, bk, Wv, bv, Bbias):
    """Host-side sharding + layout prep. Returns (in_maps, with_bias)."""
    x = np.asarray(x, np.float32)
    with_bias = bool(np.any(bq) or np.any(bk))
    if np.any(bv):
        raise NotImplementedError("nonzero bv not supported")
    wq16 = np.ascontiguousarray(np.asarray(Wq, np.float32).T / 8.0).astype(np.float16)
    wk16 = np.ascontiguousarray(np.asarray(Wk, np.float32).T).astype(np.float16)
    wv16 = np.ascontiguousarray(np.asarray(Wv, np.float32).T).astype(np.float16)
    bb = np.asarray(Bbias, np.float32).T
    ebt = np.tile(np.concatenate([bb, bb], 0), (1, 8)).astype(np.float16)
    # [128 (k x2), 8 h x 64 q]: raw Bbias^T for PSUM preload
    i64 = np.eye(64, dtype=np.float16)
    iit = np.concatenate([i64, i64], 1)
    common = {"wq": wq16, "wk": wk16, "wv": wv16, "ebt": ebt,
              "iit": iit}
    if with_bias:
        bqk = np.concatenate(
            [np.asarray(bq, np.float32).reshape(4, 128).T / 8.0,
             np.asarray(bk, np.float32).reshape(4, 128).T], 1)  # [128, 8]
        common["bqk"] = np.ascontiguousarray(bqk)
    in_maps = []
    for b in range(B):
        xT16 = np.ascontiguousarray(
            x[b].reshape(TOK, C).T).astype(np.float16)
        in_maps.append({"xT": xT16, **common})
    return in_maps, with_bias


def kernel(**inputs):
    from concourse.bass_utils import run_bass_kernel_spmd
    in_maps, with_bias = make_in_maps(**inputs)
    nc = get_model(with_bias)
    res = run_bass_kernel_spmd(
        nc, in_maps, core_ids=list(range(B)), trace=TRACE)
    LAST["results"] = res
    out = np.stack([r["out"] for r in res.results], 0)
    return out.reshape(B, C, HH, WW).astype(np.float32)


def _harvest_io(nc):
    import jax
    in_names, out_names, out_avals = [], [], []
    part_name = nc.partition_id_tensor.name if nc.partition_id_tensor else None
    for alloc in nc.m.functions[0].allocations:
        if not isinstance(alloc, mybir.MemoryLocationSet):
            continue
        name = alloc.memorylocations[0].name
        if alloc.kind == "ExternalInput":
            if name != part_name:
                in_names.append(name)
        elif alloc.kind == "ExternalOutput":
            out_names.append(name)
            out_avals.append(jax.core.ShapedArray(
                tuple(alloc.tensor_shape), mybir.dt.np(alloc.dtype)))
    return in_names, out_names, out_avals, part_name


def _make_runner(nc, in_maps):
    """Build a jitted shard_map runner for `nc` with device-resident args.
    Returns (call, fetch): call() runs once and blocks; fetch() downloads
    the last call's per-core outputs."""
    import jax
    from jax.sharding import Mesh, PartitionSpec
    from jax.experimental.shard_map import shard_map
    from concourse import bass2jax

    bass2jax.install_neuronx_cc_hook()
    in_names, out_names, out_avals, part_name = _harvest_io(nc)
    n_params = len(in_names)
    all_names = tuple(in_names + out_names
                      + ([part_name] if part_name else []))
    n_cores = len(in_maps)

    def _body(*args):
        pid = ([bass2jax.partition_id_tensor()] if part_name else [])
        return tuple(bass2jax._bass_exec_p.bind(
            *args, *pid,
            out_avals=tuple(out_avals),
            in_names=all_names,
            out_names=tuple(out_names),
            lowering_input_output_aliases=(),
            sim_require_finite=True,
            sim_require_nnan=True,
            nc=nc))

    devices = jax.devices()[:n_cores]
    mesh = Mesh(np.asarray(devices), ("core",))
    n_all = n_params + len(out_names)
    sharded = jax.jit(shard_map(
        _body, mesh=mesh,
        in_specs=(PartitionSpec("core"),) * n_all,
        out_specs=(PartitionSpec("core"),) * len(out_names),
        check_rep=False), keep_unused=True)
    concat_in = [
        np.concatenate([np.asarray(m[name]) for m in in_maps], 0)
        for name in in_names]
    concat_zeros = [
        np.zeros((n_cores * a.shape[0], *a.shape[1:]), a.dtype)
        for a in out_avals]
    args = [jax.device_put(a) for a in concat_in + concat_zeros]
    state = {}

    def call():
        import time
        t0 = time.perf_counter()
        out = sharded(*args)
        jax.block_until_ready(out)
        state["out"] = out
        return time.perf_counter() - t0

    def fetch():
        out = state["out"]
        return [
            {name: np.asarray(out[i]).reshape(n_cores, *out_avals[i].shape)[c]
             for i, name in enumerate(out_names)}
            for c in range(n_cores)]

    return call, fetch


def _min_time(call, warmup=3, iters=30):
    for _ in range(warmup):
        call()
    ts = sorted(call() for _ in range(iters))
    return ts[0], ts[len(ts) // 2]


def _make_null_runner():
    """Minimal NEFF with the same dispatch path — measures the per-call
    axon/PJRT floor to subtract from the reps model."""
    from contextlib import ExitStack as _ES
    nc = bass.Bass("TRN2", target_bir_lowering=False, debug=False)
    xT = nc.dram_tensor("xT", [512, 4096], F16, kind="ExternalInput").ap()
    out = nc.dram_tensor("out", [4096, 512], F16, kind="ExternalOutput").ap()
    with tile.TileContext(nc) as tc:
        with _ES() as ctx:
            sb = ctx.enter_context(tc.tile_pool(name="sb", bufs=1))
            t = sb.tile([128, 512], F16, tag="t")
            nc.sync.dma_start(t[:], xT[0:128, 0:512])
            nc.sync.dma_start(out[0:128, :], t[:])
    _legalize_sync(nc)
    return _make_runner(nc, [{"xT": np.zeros((512, 4096), np.float16)}])[0]


def time_kernel(inputs, reps=48, iters=15, rounds=3):
    """Returns (ns_per_iter, output). Per-call axon dispatch overhead is
    measured with a null NEFF (interleaved min-of-N) and subtracted from a
    single-core `reps`-body NEFF; per-iteration time is the difference /
    reps. The 8-core SPMD output comes from a separate 1-rep run."""
    in_maps, with_bias = make_in_maps(**inputs)
    call1, fetch1 = _make_runner(get_model(with_bias), in_maps)
    call1()
    out = np.stack([r["out"] for r in fetch1()], 0).reshape(B, C, HH, WW)
    out = np.asarray(out, np.float32)
    null_call = _make_null_runner()
    callR, _ = _make_runner(get_model(with_bias, reps=reps), in_maps[:1])
    best_null, best_r = np.inf, np.inf
    for r in range(rounds):
        best_null = min(best_null, *[null_call() for _ in range(iters)])
        best_r = min(best_r, *[callR() for _ in range(iters)])
        print(f"  [timing] round {r}: null {best_null*1e3:.2f} ms, "
              f"{reps}-rep {best_r*1e3:.2f} ms -> "
              f"{(best_r-best_null)/reps*1e6:.1f} us/iter", flush=True)
    ns = (best_r - best_null) / reps * 1e9
    return ns, out



# revision 12
# speedup vs baseline: 1.7610x; 1.7610x over previous
"""Windowed multi-head attention TRN2 kernel (Bass/Tile), SPMD over 8 cores.

Problem (per reference): x:(8,512,64,64) viewed as (B, 4096 tok, 512 c);
Q/K/V = tok @ W^T + b; per window (64 tok) & head (8 x 64d):
softmax(QK^T/8 + Bbias) @ V; output back in (B,512,64,64).

Sharding: data-parallel, one batch element per core (8 cores).

Per-core dataflow (fp16 matmul operands, fp32 PSUM accum, fp16 HBM out):
 - host passes x^T (c, tok) fp16; one 3-D-AP DMA loads all 4 channel
   chunks of a 512-token tile
 - Q^T/K^T projections evacuate naturally on ACT, then DVE strided
   copies build per-head STACKED (Qs) and BLOCK-DIAGONAL (Kd) operands;
   off-diagonal zeros live in persistent tiles, written once (all
   partition-shifted copies run on DVE — NEVER GpSimd/Pool: its 8 DSPs
   own fixed 16-partition slices and cross-partition traffic crawls)
 - scores: per (window-pair, head) ONE full-128 matmul
   s[:, h] = Kd_h^T @ Qs_h stacks both windows' scores^T k-major;
   Bbias^T is pre-seeded into the PSUM bank by an [I|I]^T @ BbiasT
   matmul (start=True) and QK accumulates on top (start=False,
   stop only on the last head - stop clears the whole zero-region)
 - exp on ACT writes straight into the pre-zeroed block-diag ptd tile
   (two strided half-partition activations)
 - PV: ONE full-128 matmul per head: o = ptd_h^T @ V_h; V natural
   [tok, c] with a per-head ones-column (65-wide blocks, written once
   into persistent tiles) so PV also emits softmax denominators
 - scores(tt+1), scores(tt+2) are issued on the PE before PV(tt)
   (software pipelining) so ACT exp runs in the PE queue's shadow
 - normalize: per 4-head group one strided reciprocal + one strided
   broadcast multiply on DVE; fp16 [tok, c] tiles stored to HBM
"""

import sys
import numpy as np

for _p in ("/opt/trn_rl_repo",):
    if _p not in sys.path:
        sys.path.insert(0, _p)

from contextlib import ExitStack

import concourse.bass as bass
import concourse.tile as tile
from concourse import mybir

F16 = mybir.dt.float16
F32 = mybir.dt.float32

B, C, HH, WW = 8, 512, 64, 64
NH, HD = 8, 64
WIN = 64            # tokens per window
TOK = C * 0 + 4096  # tokens per batch/core
NT = 8              # 512-token tiles per core
NCHUNK = 4          # 128-channel chunks

TRACE = False
LAST = {}


def _emit(tc, out, xT, wq, wk, wv, ebt, iit, bqk):
    """Emit the per-core program. bqk: [128, 8] fp32 (bq/8 | bk chunks) or None.

    Block-diagonal dataflow: per (window-pair tt, head h), scores and PV are
    ONE full-128-partition matmul each. The stationary operands are built
    block-diagonally (zeros off-diagonal, pre-zeroed once per pool buffer at
    startup, diagonal blocks rewritten each iteration):
      Kd_h  [128 ch|ch, 4tt x (k(w0)|k(w1))]  - K^T diag per window pair
      Qs_h  [128 ch|ch, 4tt x 64q]            - Q^T stacked (w0 top, w1 bot)
      ptd   [128 k|k,  8h x (q(w0)|q(w1))]    - exp(scores^T)*ebt diag
    so  s[:, h] = Kd_h(tt)^T @ Qs_h(tt)  gives both windows' scores^T stacked
    and o[:, hh] = ptd(h)^T @ V(h) gives both windows' outputs stacked —
    token-major, normalized by one batched reciprocal+mul per 4-head group.
    """
    nc = tc.nc
    Exp = mybir.ActivationFunctionType.Exp
    Ident = mybir.ActivationFunctionType.Identity

    with ExitStack() as ctx:
        ep = ctx.enter_context

        wpool = ep(tc.tile_pool(name="w", bufs=1))
        xpool = ep(tc.tile_pool(name="x", bufs=2))
        qkpool = ep(tc.tile_pool(name="qk", bufs=2))
        vpool = ep(tc.tile_pool(name="v", bufs=2))
        epool = ep(tc.tile_pool(name="e", bufs=2))
        rcpool = ep(tc.tile_pool(name="rc", bufs=4))
        onpool = ep(tc.tile_pool(name="on", bufs=3))
        projps = ep(tc.tile_pool(name="projps", bufs=3, space="PSUM"))
        sps = ep(tc.tile_pool(name="sps", bufs=3, space="PSUM"))
        ops = ep(tc.tile_pool(name="ops", bufs=2, space="PSUM"))

        # resident weights: [c_in chunk 128, c_out 512] fp16 per proj
        wsb = {}
        for nm, wdram in (("q", wq), ("k", wk), ("v", wv)):
            for ci in range(NCHUNK):
                t = wpool.tile([128, 512], F16, tag=f"w{nm}{ci}")
                nc.sync.dma_start(t[:], wdram[ci * 128:(ci + 1) * 128, :])
                wsb[nm, ci] = t
        # ebt here is raw Bbias^T (k-rows duplicated) tiled across the 8
        # head column-blocks: [128, 512] f16, preloaded into the scores
        # PSUM bank each subtile so the QK matmuls accumulate on top of it.
        ebt_sb = wpool.tile([128, 512], F16, tag="ebt")
        nc.sync.dma_start(ebt_sb[:], ebt[:, :])
        bqk_sb = None
        if bqk is not None:
            bqk_sb = wpool.tile([128, 8], F32, tag="bqk")
            nc.sync.dma_start(bqk_sb[:], bqk[:, :])

        # persistent double-buffered block-diagonal tiles (allocated once so
        # the zero regions survive across iterations; diagonal blocks are
        # rewritten each use, off-diagonal zeros written once here).
        kdbuf = [wpool.tile([128, 4096], F16, tag=f"kd{b}", name=f"kd{b}")
                 for b in range(2)]
        qsbuf = [wpool.tile([128, 2048], F16, tag=f"qs{b}", name=f"qs{b}")
                 for b in range(2)]
        ptdbuf = [wpool.tile([128, 1024], F16, tag=f"ptd{b}",
                             name=f"ptd{b}")
                  for b in range(3)]
        for b in range(2):
            nc.vector.memset(kdbuf[b][:], 0.0)
        for b in range(3):
            nc.vector.memset(ptdbuf[b][:], 0.0)

        # [I64 | I64] stationary: one matmul per subtile seeds the scores
        # PSUM bank with the Bbias^T pattern (out[k, h*64+q] = ebt[k%64, q])
        iit_sb = wpool.tile([64, 128], F16, tag="iit")
        nc.sync.dma_start(iit_sb[:], iit[:, :])

        # persistent V tiles: the per-head ones column (col 64 of each
        # 65-block) is written once here and never touched again; the
        # in-loop evacuation only writes cols 0:64 of each block.
        vnbuf = [[wpool.tile([128, 520], F16, tag=f"vn{b}{tt}",
                             name=f"vn{b}{tt}")
                  for tt in range(NCHUNK)] for b in range(2)]
        for b in range(2):
            for tt in range(NCHUNK):
                vv = vnbuf[b][tt][:].rearrange("p (h x) -> p h x", x=65)
                nc.scalar.activation(
                    vv[:, :, 64], ebt_sb[:, 0:8], Ident,
                    bias=1.0, scale=0.0)

        for T in range(NT):
            # ---- load all four x^T chunks [c_in 128, tok 512] in one DMA
            xt_all = xpool.tile([128, 2048], F16, tag="xt")
            nc.sync.dma_start(
                xt_all[:].rearrange("p (c t) -> p c t", t=512),
                xT[:, T * 512:(T + 1) * 512].rearrange(
                    "(c p) t -> p c t", p=128))


            # ---- Q^T / K^T projections: natural full-width ACT evacuation,
            # then f16 strided DVE copies into the stacked / diagonal
            # layouts.
            qsa, kda = qsbuf[T % 2], kdbuf[T % 2]
            for pi, nm in enumerate(("q", "k")):
                nat = qkpool.tile([128, 2048], F16, tag=f"{nm}nat")
                for co in range(NCHUNK):
                    ps = projps.tile([128, 512], F32, tag="proj")
                    for ci in range(NCHUNK):
                        nc.tensor.matmul(
                            ps[:],
                            wsb[nm, ci][:, co * 128:(co + 1) * 128],
                            xt_all[:, ci * 512:(ci + 1) * 512],
                            start=(ci == 0), stop=(ci == NCHUNK - 1))
                    dst = nat[:, co * 512:(co + 1) * 512]
                    if bqk_sb is not None:
                        nc.scalar.activation(
                            dst, ps[:], Ident,
                            bias=bqk_sb[:, pi * 4 + co:pi * 4 + co + 1])
                    else:
                        nc.scalar.copy(dst, ps[:])
                # merged strided copies: all 4 channel chunks in one AP.
                # nat cols viewed as (co, tt, parity, 64tok)
                n_v = nat[:].rearrange(
                    "p (c t two x) -> p c t two x", c=4, two=2, x=64)
                qs_v = qsa[:].rearrange(
                    "p (c u t x) -> p c u t x", c=4, u=2, x=64)
                kd_v = kda[:].rearrange(
                    "p (c u t y x) -> p c u t y x", c=4, u=2, y=2, x=64)
                for e in range(2):
                    er = slice(e * 64, e * 64 + 64)
                    for p in range(2):
                        pr = slice(p * 64, p * 64 + 64)
                        if nm == "q":
                            nc.vector.tensor_copy(
                                qs_v[pr, :, e, :, :], n_v[er, :, :, p, :])
                        else:
                            nc.vector.tensor_copy(
                                kd_v[pr, :, e, :, p, :], n_v[er, :, :, p, :])

            # ---- V natural projection per 128-tok subtile -> [tok 128,
            # 8 x (64 d | 1)] with a ones column per head for softmax sums
            vnat = []
            for tt in range(NCHUNK):
                ps = projps.tile([128, 512], F32, tag="proj")
                for ci in range(NCHUNK):
                    nc.tensor.matmul(
                        ps[:],
                        xt_all[:, ci * 512 + tt * 128:
                               ci * 512 + (tt + 1) * 128],
                        wsb["v", ci][:],
                        start=(ci == 0), stop=(ci == NCHUNK - 1))
                vn = vnbuf[T % 2][tt]
                vn_v = vn[:].rearrange("p (h x) -> p h x", x=65)
                nc.scalar.copy(
                    vn_v[:, :, 0:64],
                    ps[:].rearrange("p (h x) -> p h x", x=64))
                vnat.append(vn)

            # ---- attention: subtile tt covers windows 2tt (partitions
            # 0:64) and 2tt+1 (64:128); one full-128 matmul per head for
            # scores and for PV via the block-diagonal stationaries.
            # Software-pipelined: scores(tt+1), scores(tt+2) are issued on
            # the PE BEFORE PV(tt), so the exp stage (ACT, written straight
            # into the diag tile) runs in the shadow of PE work instead of
            # stalling the in-order PE queue at every handoff.
            def emit_scores(tt):
                s = sps.tile([128, 512], F32, tag="s")
                # seed full bank with Bbias^T, then accumulate QK on top
                nc.tensor.matmul(s[:], iit_sb[:], ebt_sb[0:64, :],
                                 start=True, stop=False)
                for h in range(8):
                    nc.tensor.matmul(
                        s[:, h * 64:(h + 1) * 64],
                        kda[:, h * 512 + tt * 128:
                            h * 512 + (tt + 1) * 128],
                        qsa[:, h * 256 + tt * 64:
                            h * 256 + (tt + 1) * 64],
                        start=False, stop=(h == 7))
                ptd = ptdbuf[tt % 3]
                ptd_v = ptd[:].rearrange("p (h x) -> p h x", x=128)
                for p in range(2):
                    r = slice(p * 64, p * 64 + 64)
                    nc.scalar.activation(
                        ptd_v[r, :, p * 64:(p + 1) * 64],
                        s[r, :].rearrange("p (h x) -> p h x", x=64), Exp)
                return ptd

            ptds = [emit_scores(0), emit_scores(1)]
            for tt in range(NCHUNK):
                if tt + 2 < NCHUNK:
                    ptds.append(emit_scores(tt + 2))
                ptd = ptds[tt]
                # PV into two 4-head PSUM tiles [128, 4 x (64 d | 1 sum)]
                on = onpool.tile([128, 512], F16, tag="on")
                rc = rcpool.tile([128, 8], F32, tag="rc")
                for g in range(2):
                    o = ops.tile([128, 512], F32, tag="o")
                    o_v = o[:, 0:260].rearrange("p (h x) -> p h x", x=65)
                    for hh in range(4):
                        h = g * 4 + hh
                        nc.tensor.matmul(
                            o[:, hh * 65:(hh + 1) * 65],
                            ptd[:, h * 128:(h + 1) * 128],
                            vnat[tt][:, h * 65:(h + 1) * 65],
                            start=True, stop=True)
                    nc.vector.reciprocal(
                        rc[:, g * 4:(g + 1) * 4], o_v[:, :, 64])
                    nc.vector.tensor_mul(
                        on[:, g * 256:(g + 1) * 256].rearrange(
                            "p (h x) -> p h x", x=64),
                        o_v[:, :, 0:64],
                        rc[:, g * 4:(g + 1) * 4].unsqueeze(2).broadcast_to(
                            (128, 4, 64)))
                nc.sync.dma_start(
                    out[T * 512 + tt * 128: T * 512 + (tt + 1) * 128, :], on[:])


def _legalize_sync(nc, max_waits=1):
    """Hoist excess semaphore waits into standalone same-engine
    EventSemaphore instructions. Engine instruction streams execute in
    order, so a wait carried by an immediately-preceding EventSemaphore is
    equivalent to a wait on the instruction itself — and the walrus build
    in this environment rejects instructions with more than one wait."""
    import bass_rust
    n_new = 0
    fn = nc.m.functions[0]
    for blk in fn.blocks:
        out = []
        changed = False
        for ins in blk.instructions:
            si = ins.sync_info
            waits = list(si.on_wait) if si and si.on_wait else []
            if len(waits) > max_waits:
                keep = waits[-max_waits:]
                for w in waits[:-max_waits]:
                    es = mybir.InstEventSemaphore(
                        name=f"esw-{n_new}-{ins.name}", ins=[], outs=[])
                    es.engine = ins.engine
                    es.sync_info = bass_rust.SyncInfo(on_wait=[w], on_update=[])
                    out.append(es)
                    n_new += 1
                ins.sync_info = bass_rust.SyncInfo(
                    on_wait=keep,
                    on_update=list(si.on_update) if si.on_update else [])
                changed = True
            out.append(ins)
        if changed:
            blk.instructions = out
    return n_new


def _build_model(with_bias, reps=1):
    nc = bass.Bass("TRN2", target_bir_lowering=False, debug=False)
    xT = nc.dram_tensor("xT", [512, 4096], F16, kind="ExternalInput").ap()
    wq = nc.dram_tensor("wq", [512, 512], F16, kind="ExternalInput").ap()
    wk = nc.dram_tensor("wk", [512, 512], F16, kind="ExternalInput").ap()
    wv = nc.dram_tensor("wv", [512, 512], F16, kind="ExternalInput").ap()
    ebt = nc.dram_tensor("ebt", [128, 512], F16, kind="ExternalInput").ap()
    iit = nc.dram_tensor("iit", [64, 128], F16, kind="ExternalInput").ap()
    bqk = (nc.dram_tensor("bqk", [128, 8], F32, kind="ExternalInput").ap()
           if with_bias else None)
    outs = [nc.dram_tensor("out" if r == 0 else f"out{r}", [4096, 512], F16,
                           kind="ExternalOutput").ap() for r in range(reps)]
    with tile.TileContext(nc) as tc:
        for r in range(reps):
            _emit(tc, outs[r], xT, wq, wk, wv, ebt, iit, bqk)
    return nc


_MODEL_CACHE = {}


def get_model(with_bias=False, legalize=True, reps=1):
    key = (with_bias, legalize, reps)
    if key not in _MODEL_CACHE:
        nc = _build_model(with_bias, reps=reps)
        if legalize:
            _legalize_sync(nc)
        _MODEL_CACHE[key] = nc
    return _MODEL_CACHE[key]


def make_in_maps(x, Wq, bq, Wk, bk, Wv, bv, Bbias):
    """Host-side sharding + layout prep. Returns (in_maps, with_bias)."""
    x = np.asarray(x, np.float32)
    with_bias = bool(np.any(bq) or np.any(bk))
    if np.any(bv):
        raise NotImplementedError("nonzero bv not supported")
    wq16 = np.ascontiguousarray(np.asarray(Wq, np.float32).T / 8.0).astype(np.float16)
    wk16 = np.ascontiguousarray(np.asarray(Wk, np.float32).T).astype(np.float16)
    wv16 = np.ascontiguousarray(np.asarray(Wv, np.float32).T).astype(np.float16)
    bb = np.asarray(Bbias, np.float32).T
    ebt = np.tile(np.concatenate([bb, bb], 0), (1, 8)).astype(np.float16)
    # [128 (k x2), 8 h x 64 q]: raw Bbias^T for PSUM preload
    i64 = np.eye(64, dtype=np.float16)
    iit = np.concatenate([i64, i64], 1)
    common = {"wq": wq16, "wk": wk16, "wv": wv16, "ebt": ebt,
              "iit": iit}
    if with_bias:
        bqk = np.concatenate(
            [np.asarray(bq, np.float32).reshape(4, 128).T / 8.0,
             np.asarray(bk, np.float32).reshape(4, 128).T], 1)  # [128, 8]
        common["bqk"] = np.ascontiguousarray(bqk)
    in_maps = []
    for b in range(B):
        xT16 = np.ascontiguousarray(
            x[b].reshape(TOK, C).T).astype(np.float16)
        in_maps.append({"xT": xT16, **common})
    return in_maps, with_bias


def kernel(**inputs):
    from concourse.bass_utils import run_bass_kernel_spmd
    in_maps, with_bias = make_in_maps(**inputs)
    nc = get_model(with_bias)
    res = run_bass_kernel_spmd(
        nc, in_maps, core_ids=list(range(B)), trace=TRACE)
    LAST["results"] = res
    out = np.stack([r["out"] for r in res.results], 0)
    return out.reshape(B, C, HH, WW).astype(np.float32)


def _harvest_io(nc):
    import jax
    in_names, out_names, out_avals = [], [], []
    part_name = nc.partition_id_tensor.name if nc.partition_id_tensor else None
    for alloc in nc.m.functions[0].allocations:
        if not isinstance(alloc, mybir.MemoryLocationSet):
            continue
        name = alloc.memorylocations[0].name
        if alloc.kind == "ExternalInput":
            if name != part_name:
                in_names.append(name)
        elif alloc.kind == "ExternalOutput":
            out_names.append(name)
            out_avals.append(jax.core.ShapedArray(
                tuple(alloc.tensor_shape), mybir.dt.np(alloc.dtype)))
    return in_names, out_names, out_avals, part_name


def _make_runner(nc, in_maps):
    """Build a jitted shard_map runner for `nc` with device-resident args.
    Returns (call, fetch): call() runs once and blocks; fetch() downloads
    the last call's per-core outputs."""
    import jax
    from jax.sharding import Mesh, PartitionSpec
    from jax.experimental.shard_map import shard_map
    from concourse import bass2jax

    bass2jax.install_neuronx_cc_hook()
    in_names, out_names, out_avals, part_name = _harvest_io(nc)
    n_params = len(in_names)
    all_names = tuple(in_names + out_names
                      + ([part_name] if part_name else []))
    n_cores = len(in_maps)

    def _body(*args):
        pid = ([bass2jax.partition_id_tensor()] if part_name else [])
        return tuple(bass2jax._bass_exec_p.bind(
            *args, *pid,
            out_avals=tuple(out_avals),
            in_names=all_names,
            out_names=tuple(out_names),
            lowering_input_output_aliases=(),
            sim_require_finite=True,
            sim_require_nnan=True,
            nc=nc))

    devices = jax.devices()[:n_cores]
    mesh = Mesh(np.asarray(devices), ("core",))
    n_all = n_params + len(out_names)
    sharded = jax.jit(shard_map(
        _body, mesh=mesh,
        in_specs=(PartitionSpec("core"),) * n_all,
        out_specs=(PartitionSpec("core"),) * len(out_names),
        check_rep=False), keep_unused=True)
    concat_in = [
        np.concatenate([np.asarray(m[name]) for m in in_maps], 0)
        for name in in_names]
    concat_zeros = [
        np.zeros((n_cores * a.shape[0], *a.shape[1:]), a.dtype)
        for a in out_avals]
    args = [jax.device_put(a) for a in concat_in + concat_zeros]
    state = {}

    def call():
        import time
        t0 = time.perf_counter()
        out = sharded(*args)
        jax.block_until_ready(out)
        state["out"] = out
        return time.perf_counter() - t0

    def fetch():
        out = state["out"]
        return [
            {name: np.asarray(out[i]).reshape(n_cores, *out_avals[i].shape)[c]
             for i, name in enumerate(out_names)}
            for c in range(n_cores)]

    return call, fetch


def _min_time(call, warmup=3, iters=30):
    for _ in range(warmup):
        call()
    ts = sorted(call() for _ in range(iters))
    return ts[0], ts[len(ts) // 2]


def _make_null_runner():
    """Minimal NEFF with the same dispatch path — measures the per-call
    axon/PJRT floor to subtract from the reps model."""
    from contextlib import ExitStack as _ES
    nc = bass.Bass("TRN2", target_bir_lowering=False, debug=False)
    xT = nc.dram_tensor("xT", [512, 4096], F16, kind="ExternalInput").ap()
    out = nc.dram_tensor("out", [4096, 512], F16, kind="ExternalOutput").ap()
    with tile.TileContext(nc) as tc:
        with _ES() as ctx:
            sb = ctx.enter_context(tc.tile_pool(name="sb", bufs=1))
            t = sb.tile([128, 512], F16, tag="t")
            nc.sync.dma_start(t[:], xT[0:128, 0:512])
            nc.sync.dma_start(out[0:128, :], t[:])
    _legalize_sync(nc)
    return _make_runner(nc, [{"xT": np.zeros((512, 4096), np.float16)}])[0]


def time_kernel(inputs, reps=48, iters=15, rounds=3):
    """Returns (ns_per_iter, output). Per-call axon dispatch overhead is
    measured with a null NEFF (interleaved min-of-N) and subtracted from a
    single-core `reps`-body NEFF; per-iteration time is the difference /
    reps. The 8-core SPMD output comes from a separate 1-rep run."""
    in_maps, with_bias = make_in_maps(**inputs)
    call1, fetch1 = _make_runner(get_model(with_bias), in_maps)
    call1()
    out = np.stack([r["out"] for r in fetch1()], 0).reshape(B, C, HH, WW)
    out = np.asarray(out, np.float32)
    null_call = _make_null_runner()
    callR, _ = _make_runner(get_model(with_bias, reps=reps), in_maps[:1])
    best_null, best_r = np.inf, np.inf
    for r in range(rounds):
        best_null = min(best_null, *[null_call() for _ in range(iters)])
        best_r = min(best_r, *[callR() for _ in range(iters)])
        print(f"  [timing] round {r}: null {best_null*1e3:.2f} ms, "
              f"{reps}-rep {best_r*1e3:.2f} ms -> "
              f"{(best_r-best_null)/reps*1e6:.1f} us/iter", flush=True)
    ns = (best_r - best_null) / reps * 1e9
    return ns, out

